# revision 11
# baseline (speedup 1.0000x reference)
"""AcidBaseDense Trainium2 kernel (centered fp8 DoubleRow + host log epilogue).

Math (reference, f32):
    bw   = sign(clip(w, -1, 1))                    in {-1, +1} (w ~ N(0,1))
    h    = 10^(-x);  oh = 1e-14 / h                (oh <= 1e-13 << f32 eps of h)
    r    = (h*0.1) @ bw - (oh*0.1) @ bw            == (h*0.1) @ bw  at f32 precision
    ph   = 7 + sign(r) * (-log10(|r|/409.6) - 7)

Split of work:
  * device (the 137 GFLOP matmul, 99.97% of the FLOPs):
      a' = 64*10^(-x) via ACT Exp (u8 in -> fp16),
      d  = fp8e4m3(a' - 25)        (DVE; centering nearly halves the e4m3
                                    quantization noise: sigma_dr 0.045 vs
                                    0.079 in r' units/640),
      r' = d @ sign(w) + 25*colsum(sign(w)) -- matmul on PE with
      perf_mode=DoubleRow (K=256/matmul, 2 MACs/cell/cycle -- 2x the fp16
      roofline the v1 kernel was pinned at); the colsum term is an exact
      integer constant per output column, added on host.
      r16 = fp16(psum) copied out.  No device log epilogue: avoids the
      1.3us ACT table reload per Exp<->Ln switch, 4 DVE ops/chunk, and
      shortens teardown.
  * host: the O(B*N) log epilogue ph(r), an exact colsum shift, and a
    f64-grade fixup of the near-cancellation band |r| < T_BAND where the
    quantization noise is log-amplified (~3-4%% of outputs; products h*s
    are exact in f32 since s = +-1, summed pairwise in f64).
  * k mapping: k = kc*256 + i*128 + p; host pre-tiles operands so every DMA
    is a flat multi-KB per-partition run.  s8 is chunked so the m0 chain can
    start before the full 4MB arrives; m0/m15 run their two n-chains
    sequentially (m0: only s8-half0 resident early; m15: drain overlaps),
    others interleave both chains per kc to share LDWEIGHTS.
"""

import os
import sys

for _p in ("/opt/trn_rl_repo", "/root/.axon_site/_ro/trn_rl_repo"):
    if os.path.isdir(_p) and _p not in sys.path:
        sys.path.insert(0, _p)

import numpy as np

BATCH = 4096
N_IN = 4096
N_OUT = 4096
B_GROUPS = 2           # batch shards
N_GROUPS = 4           # n_out shards
B_SH = BATCH // B_GROUPS      # 2048 batch rows per core
N_SH = N_OUT // N_GROUPS      # 1024 out cols per core
KC = N_IN // 256              # 16 doublerow contraction chunks
MT = B_SH // 128              # 16 batch tiles per core
NCHUNK = 2                    # two 512-wide PSUM chunks per batch tile

LN10 = float(np.log(10.0))
XQ = 255.0                    # u8 x quantization
ASCALE = 64.0                 # a' = 64 * 10^-x ;  r' = 640 * r
CENTER = 25.0                 # quantize a' - CENTER in e4m3
EXP_SCALE = float(-LN10 / XQ)
EXP_BIAS = float(np.log(ASCALE))
RSCALE = ASCALE * 10.0        # r' = RSCALE * r
T_BAND = 0.135                # |r| < T_BAND recomputed precisely on host

_CACHED = {}


def _build_nc():
    import concourse.bacc as bacc
    import concourse.mybir as mybir
    import concourse.tile as tile

    F32 = mybir.dt.float32
    FP16 = mybir.dt.float16
    FP8 = mybir.dt.float8e4
    U8 = mybir.dt.uint8
    AFT = mybir.ActivationFunctionType
    ALU = mybir.AluOpType
    DR = mybir.MatmulPerfMode.DoubleRow

    nc = bacc.Bacc(trn_type="TRN2")
    # x^T pre-tiled on host: [m_tile, partition p, kc, i, mb] with
    # k = kc*256 + i*128 + p, b = m*128 + mb
    xt_d = nc.dram_tensor("xt", [MT, 128, KC, 2, 128], U8, kind="ExternalInput")
    # s pre-tiled: [nch, partition p, kc, i, nn] with n = nch*512 + nn
    s8_d = nc.dram_tensor("s8", [NCHUNK, 128, KC, 2, 512], FP8, kind="ExternalInput")
    r_d = nc.dram_tensor("r16", [B_SH, N_SH], FP16, kind="ExternalOutput")

    with tile.TileContext(nc) as tc:
        with (
            tc.tile_pool(name="spool", bufs=1) as spool,
            tc.tile_pool(name="mpool", bufs=3) as mpool,
            tc.tile_pool(name="epool", bufs=3) as epool,
            tc.tile_pool(name="cpool", bufs=1) as cpool,
            tc.tile_pool(name="ph_pool", bufs=6, space="PSUM") as ph_pool,
        ):
            cf32 = cpool.tile([128, 1], F32, tag="cf32")
            nc.gpsimd.memset(cf32[:, 0:1], EXP_BIAS)
            bias_exp = cf32[:, 0:1]

            # warm-up scratch: preload the Exp ACT table and keep the PE HAM
            # clock-gate busy while the startup DMAs are in flight
            wsrc = cpool.tile([128, 2, 512], FP8, tag="wsrc")
            nc.gpsimd.memset(wsrc[:].bitcast(U8), 0)
            wact = cpool.tile([128, 1], FP16, tag="wact")
            nc.scalar.activation(
                wact[:], cf32[:, 0:1], AFT.Exp, bias=bias_exp, scale=0.0,
            )

            # m-tile prep: u8 x chunk DMA, ACT Exp to fp16, DVE center to fp8
            def load_x(m, q, nprep):
                qk = KC // nprep
                xq = mpool.tile([128, qk, 2, 128], U8, tag=f"xq{q}_{nprep}")
                nc.sync.dma_start(xq[:], xt_d[m, :, q * qk:(q + 1) * qk, :, :])
                return xq

            def exp_x(xq, q, nprep):
                qk = KC // nprep
                a16 = mpool.tile([128, qk, 2, 128], FP16, tag=f"a16q{q}_{nprep}")
                nc.scalar.activation(
                    a16[:], xq[:], AFT.Exp,
                    bias=bias_exp, scale=EXP_SCALE,
                )
                d8 = mpool.tile([128, qk, 2, 128], FP8, tag=f"d8q{q}_{nprep}")
                nc.vector.tensor_scalar(
                    d8[:], a16[:], CENTER, None, op0=ALU.subtract,
                )
                return d8

            def prep(m, nprep):
                xs = [load_x(m, q, nprep) for q in range(nprep)]
                return [exp_x(x, q, nprep) for q, x in enumerate(xs)], KC // nprep

            # S: host-binarized sign(w), exact in fp8.  One tile per n-half,
            # loaded in kc-order chunks so the m0 chain starts progressively.
            s8t = [
                spool.tile([128, KC, 2, 512], FP8, tag=f"s8_{h}", name=f"s8_{h}")
                for h in range(NCHUNK)
            ]

            def load_s(h, q, nchunks=8):
                tq = KC // nchunks
                nc.gpsimd.dma_start(
                    s8t[h][:, q * tq:(q + 1) * tq, :, :],
                    s8_d[h, :, q * tq:(q + 1) * tq, :, :],
                )

            def mm(pt, a_pack, n, kc):
                a_q, kq = a_pack
                q, j = divmod(kc, kq)
                nc.tensor.matmul(
                    pt,
                    a_q[q][:, j],
                    s8t[n][:, kc],
                    start=(kc == 0),
                    stop=(kc == KC - 1),
                    perf_mode=DR,
                    skip_group_check=True,
                )

            def drain(n, r_sb, pt, m, dma_now):
                nc.vector.tensor_copy(
                    r_sb[:, n * 512:(n + 1) * 512], pt[:]
                )
                if dma_now:
                    nc.gpsimd.dma_start(
                        r_d[m * 128:(m + 1) * 128,
                            n * 512:(n + 1) * 512],
                        r_sb[:, n * 512:(n + 1) * 512],
                    )

            # ---- pipeline.  m0's x and s8-half0 in fine chunks so the first
            # chain starts as soon as the first kc slices land.
            m0_x = [load_x(0, q, 8) for q in range(8)]
            for q in range(8):
                load_s(0, q)
            # PE warm-up: dummy matmuls on the scratch tile run during the
            # DMA wait, lifting the HAM clock gate to 8/8 before the chain
            wp = ph_pool.tile([128, 512], F32, tag="warm", bufs=1)
            for _ in range(10):
                nc.tensor.matmul(
                    wp[:], wsrc[:, :, 0:128], wsrc[:],
                    start=True, stop=True,
                    perf_mode=DR, skip_group_check=True,
                )
            a_prev = ([exp_x(x, q, 8) for q, x in enumerate(m0_x)], KC // 8)
            for q in range(8):
                load_s(1, q)
            for m in range(MT):
                a_pack = a_prev
                r_sb = epool.tile([128, N_SH], FP16, tag="r_sb")
                pts = [ph_pool.tile([128, 512], F32, tag="ph", name=f"ph{m}_{n}")
                       for n in range(NCHUNK)]
                seq = m == 0 or m == MT - 1
                if seq:
                    # sequential chains: m0 - only s8 half 0 resident early;
                    # m15 - drain of chain 0 overlaps chain 1
                    for n in range(NCHUNK):
                        for kc in range(KC):
                            if n == 0 and kc == 10 and m + 1 < MT:
                                a_prev = prep(m + 1, 2)
                            mm(pts[n], a_pack, n, kc)
                        drain(n, r_sb, pts[n], m, dma_now=True)
                else:
                    # interleaved: both chains share the stationary a'(kc)
                    for kc in range(KC):
                        if kc == KC // 2 and m + 1 < MT:
                            a_prev = prep(m + 1, 2)
                        for n in range(NCHUNK):
                            mm(pts[n], a_pack, n, kc)
                    for n in range(NCHUNK):
                        drain(n, r_sb, pts[n], m, dma_now=False)
                    nc.gpsimd.dma_start(
                        r_d[m * 128:(m + 1) * 128, :], r_sb
                    )

    nc.compile()
    return nc


def kernel(x: np.ndarray, w: np.ndarray) -> np.ndarray:
    import ml_dtypes
    from concourse.bass_utils import run_bass_kernel_spmd

    assert x.shape == (BATCH, N_IN) and w.shape == (N_IN, N_OUT)
    x = np.ascontiguousarray(x, dtype=np.float32)
    w = np.ascontiguousarray(w, dtype=np.float32)

    if "nc" not in _CACHED:
        _CACHED["nc"] = _build_nc()
    nc = _CACHED["nc"]

    # static weight preprocessing: sign(clip(w)), exactly representable
    s_full = np.sign(np.clip(w, -1.0, 1.0)).astype(np.float32)
    s8_full = s_full.astype(ml_dtypes.float8_e4m3)
    xu8 = np.rint(x * XQ).astype(np.uint8)

    xt_by_bg = {}
    s8_by_ng = {}
    for bg in range(B_GROUPS):
        xsh = xu8[bg * B_SH:(bg + 1) * B_SH, :]
        # [k, b] -> [m, p, kc, i, mb] with k = kc*256 + i*128 + p
        xt_by_bg[bg] = np.ascontiguousarray(
            xsh.T.reshape(KC, 2, 128, MT, 128).transpose(3, 2, 0, 1, 4)
        )
    for ng in range(N_GROUPS):
        ssh = s8_full[:, ng * N_SH:(ng + 1) * N_SH]
        # [k, n] -> [nch, p, kc, i, nn]
        s8_by_ng[ng] = np.ascontiguousarray(
            ssh.reshape(KC, 2, 128, NCHUNK, 512).transpose(3, 2, 0, 1, 4)
        )

    in_maps = []
    for c in range(8):
        bg, ng = divmod(c, N_GROUPS)
        in_maps.append({"xt": xt_by_bg[bg], "s8": s8_by_ng[ng]})

    trace = os.environ.get("PH_KERNEL_TRACE", "") == "1"
    kwargs = {"trace_cores": list(range(8))} if trace else {}
    try:
        res = run_bass_kernel_spmd(
            nc, in_maps, core_ids=list(range(8)), trace=trace, **kwargs
        )
    except Exception as e:  # transient NRT_EXEC_UNIT_UNRECOVERABLE seen rarely
        if "UNRECOVERABLE" not in str(e) and "UNAVAILABLE" not in str(e):
            raise
        import time
        time.sleep(5.0)
        res = run_bass_kernel_spmd(
            nc, in_maps, core_ids=list(range(8)), trace=trace, **kwargs
        )
    if trace:
        _CACHED["last_result"] = res

    # gather device partial r' = (d8 @ s) from the cores, add the exact
    # centering shift CENTER * colsum(s) back in
    rp = np.empty((BATCH, N_OUT), dtype=np.float32)
    for c, r in enumerate(res.results):
        bg, ng = divmod(c, N_GROUPS)
        rp[bg * B_SH:(bg + 1) * B_SH, ng * N_SH:(ng + 1) * N_SH] = (
            r["r16"].astype(np.float32)
        )
    _CACHED["rp_raw"] = rp.copy()
    colsum = s_full.sum(axis=0, dtype=np.float64)  # exact small integers
    rp = rp.astype(np.float64) + CENTER * colsum[None, :]
    r_full = rp / RSCALE

    # ---- host refinement of the near-cancellation band |r| < T_BAND:
    # recompute r = 0.1 * h . s exactly (products are +-h, exact in f32;
    # pairwise f64 sum), grouped by row to keep gather traffic low.
    flag = np.abs(r_full) < T_BAND
    if flag.any():
        h32 = np.power(10.0, -x.astype(np.float64)).astype(np.float32)
        sT8 = np.ascontiguousarray(np.sign(w).T.astype(np.int8))
        rows = np.nonzero(flag.any(axis=1))[0]
        for b in rows:
            cols = np.nonzero(flag[b])[0]
            prod = h32[b][None, :] * sT8[cols]
            r_full[b, cols] = prod.sum(axis=1, dtype=np.float64) * 0.1

    # ---- host log epilogue: ph = 7 + sign(r)*(-log10(|r|/409.6) - 7)
    t = np.abs(r_full) + 1e-300
    u = -np.log10(t) + (np.log10(409.6) - 7.0)
    y = np.where(r_full < 0, -u, u) + 7.0
    return y.astype(np.float32)


# revision 12
# speedup vs baseline: 1.0353x; 1.0353x over previous
"""AcidBaseDense Trainium2 kernel (centered fp8 DoubleRow + host log epilogue).

Math (reference, f32):
    bw   = sign(clip(w, -1, 1))                    in {-1, +1} (w ~ N(0,1))
    h    = 10^(-x);  oh = 1e-14 / h                (oh <= 1e-13 << f32 eps of h)
    r    = (h*0.1) @ bw - (oh*0.1) @ bw            == (h*0.1) @ bw  at f32 precision
    ph   = 7 + sign(r) * (-log10(|r|/409.6) - 7)

Split of work:
  * device (the 137 GFLOP matmul, 99.97% of the FLOPs):
      a' = 64*10^(-x) via ACT Exp (u8 in -> fp16),
      d  = fp8e4m3(a' - 25)        (DVE; centering nearly halves the e4m3
                                    quantization noise: sigma_dr 0.045 vs
                                    0.079 in r' units/640),
      r' = d @ sign(w) + 25*colsum(sign(w)) -- matmul on PE with
      perf_mode=DoubleRow (K=256/matmul, 2 MACs/cell/cycle -- 2x the fp16
      roofline the v1 kernel was pinned at); the colsum term is an exact
      integer constant per output column, added on host.
      r16 = fp16(psum) copied out.  No device log epilogue: avoids the
      1.3us ACT table reload per Exp<->Ln switch, 4 DVE ops/chunk, and
      shortens teardown.
  * host: the O(B*N) log epilogue ph(r), an exact colsum shift, and a
    f64-grade fixup of the near-cancellation band |r| < T_BAND where the
    quantization noise is log-amplified (~3-4%% of outputs; products h*s
    are exact in f32 since s = +-1, summed pairwise in f64).
  * k mapping: k = kc*256 + i*128 + p; host pre-tiles operands so every DMA
    is a flat multi-KB per-partition run.  s8 is chunked so the m0 chain can
    start before the full 4MB arrives; m0/m15 run their two n-chains
    sequentially (m0: only s8-half0 resident early; m15: drain overlaps),
    others interleave both chains per kc to share LDWEIGHTS.
"""

import os
import sys

for _p in ("/opt/trn_rl_repo", "/root/.axon_site/_ro/trn_rl_repo"):
    if os.path.isdir(_p) and _p not in sys.path:
        sys.path.insert(0, _p)

import numpy as np

BATCH = 4096
N_IN = 4096
N_OUT = 4096
B_GROUPS = 2           # batch shards
N_GROUPS = 4           # n_out shards
B_SH = BATCH // B_GROUPS      # 2048 batch rows per core
N_SH = N_OUT // N_GROUPS      # 1024 out cols per core
KC = N_IN // 256              # 16 doublerow contraction chunks
MT = B_SH // 128              # 16 batch tiles per core
NCHUNK = 2                    # two 512-wide PSUM chunks per batch tile

LN10 = float(np.log(10.0))
XQ = 255.0                    # u8 x quantization
ASCALE = 64.0                 # a' = 64 * 10^-x ;  r' = 640 * r
CENTER = 25.0                 # quantize a' - CENTER in e4m3
EXP_SCALE = float(-LN10 / XQ)
EXP_BIAS = float(np.log(ASCALE))
RSCALE = ASCALE * 10.0        # r' = RSCALE * r
T_BAND = 0.135                # |r| < T_BAND recomputed precisely on host

_CACHED = {}


def _build_nc():
    import concourse.bacc as bacc
    import concourse.mybir as mybir
    import concourse.tile as tile

    F32 = mybir.dt.float32
    FP16 = mybir.dt.float16
    FP8 = mybir.dt.float8e4
    U8 = mybir.dt.uint8
    AFT = mybir.ActivationFunctionType
    ALU = mybir.AluOpType
    DR = mybir.MatmulPerfMode.DoubleRow

    nc = bacc.Bacc(trn_type="TRN2")
    # x^T pre-tiled on host: [m_tile, partition p, kc, i, mb] with
    # k = kc*256 + i*128 + p, b = m*128 + mb
    xt_d = nc.dram_tensor("xt", [MT, 128, KC, 2, 128], U8, kind="ExternalInput")
    # s pre-tiled: [nch, partition p, kc, i, nn] with n = nch*512 + nn
    s8_d = nc.dram_tensor("s8", [NCHUNK, 128, KC, 2, 512], FP8, kind="ExternalInput")
    r_d = nc.dram_tensor("r16", [B_SH, N_SH], FP16, kind="ExternalOutput")

    with tile.TileContext(nc) as tc:
        with (
            tc.tile_pool(name="spool", bufs=1) as spool,
            tc.tile_pool(name="mpool", bufs=3) as mpool,
            tc.tile_pool(name="epool", bufs=3) as epool,
            tc.tile_pool(name="cpool", bufs=1) as cpool,
            tc.tile_pool(name="ph_pool", bufs=6, space="PSUM") as ph_pool,
        ):
            cf32 = cpool.tile([128, 1], F32, tag="cf32")
            nc.gpsimd.memset(cf32[:, 0:1], EXP_BIAS)
            bias_exp = cf32[:, 0:1]

            # warm-up scratch: preload the Exp ACT table and keep the PE HAM
            # clock-gate busy while the startup DMAs are in flight
            wsrc = cpool.tile([128, 2, 512], FP8, tag="wsrc")
            nc.gpsimd.memset(wsrc[:].bitcast(U8), 0)
            wact = cpool.tile([128, 1], FP16, tag="wact")
            nc.scalar.activation(
                wact[:], cf32[:, 0:1], AFT.Exp, bias=bias_exp, scale=0.0,
            )

            # m-tile prep: u8 x chunk DMA, ACT Exp to fp16, DVE center to fp8
            def load_x(m, q, nprep):
                qk = KC // nprep
                xq = mpool.tile([128, qk, 2, 128], U8, tag=f"xq{q}_{nprep}")
                nc.sync.dma_start(xq[:], xt_d[m, :, q * qk:(q + 1) * qk, :, :])
                return xq

            def exp_x(xq, q, nprep):
                qk = KC // nprep
                a16 = mpool.tile([128, qk, 2, 128], FP16, tag=f"a16q{q}_{nprep}")
                nc.scalar.activation(
                    a16[:], xq[:], AFT.Exp,
                    bias=bias_exp, scale=EXP_SCALE,
                )
                d8 = mpool.tile([128, qk, 2, 128], FP8, tag=f"d8q{q}_{nprep}")
                nc.vector.tensor_scalar(
                    d8[:], a16[:], CENTER, None, op0=ALU.subtract,
                )
                return d8

            def prep(m, nprep):
                xs = [load_x(m, q, nprep) for q in range(nprep)]
                return [exp_x(x, q, nprep) for q, x in enumerate(xs)], KC // nprep

            # S: host-binarized sign(w), exact in fp8.  One tile per n-half,
            # loaded in kc-order chunks so the m0 chain starts progressively.
            s8t = [
                spool.tile([128, KC, 2, 512], FP8, tag=f"s8_{h}", name=f"s8_{h}")
                for h in range(NCHUNK)
            ]

            def load_s(h, q, nchunks=8):
                tq = KC // nchunks
                nc.gpsimd.dma_start(
                    s8t[h][:, q * tq:(q + 1) * tq, :, :],
                    s8_d[h, :, q * tq:(q + 1) * tq, :, :],
                )

            def mm(pt, a_pack, n, kc):
                a_q, kq = a_pack
                q, j = divmod(kc, kq)
                nc.tensor.matmul(
                    pt,
                    a_q[q][:, j],
                    s8t[n][:, kc],
                    start=(kc == 0),
                    stop=(kc == KC - 1),
                    perf_mode=DR,
                    skip_group_check=True,
                )

            def drain(n, r_sb, pt, m, dma_now):
                nc.vector.tensor_copy(
                    r_sb[:, n * 512:(n + 1) * 512], pt[:]
                )
                if dma_now:
                    nc.gpsimd.dma_start(
                        r_d[m * 128:(m + 1) * 128,
                            n * 512:(n + 1) * 512],
                        r_sb[:, n * 512:(n + 1) * 512],
                    )

            # ---- pipeline.  m0's x and s8-half0 in fine chunks so the first
            # chain starts as soon as the first kc slices land.
            m0_x = [load_x(0, q, 4) for q in range(4)]
            for q in range(8):
                load_s(0, q)
            # PE warm-up: dummy matmuls on the scratch tile run during the
            # DMA wait, lifting the HAM clock gate to 8/8 before the chain
            wp = ph_pool.tile([128, 512], F32, tag="warm", bufs=1)
            for _ in range(10):
                nc.tensor.matmul(
                    wp[:], wsrc[:, :, 0:128], wsrc[:],
                    start=True, stop=True,
                    perf_mode=DR, skip_group_check=True,
                )
            a_prev = ([exp_x(x, q, 4) for q, x in enumerate(m0_x)], KC // 4)
            for q in range(8):
                load_s(1, q)
            for m in range(MT):
                a_pack = a_prev
                r_sb = epool.tile([128, N_SH], FP16, tag="r_sb")
                pts = [ph_pool.tile([128, 512], F32, tag="ph", name=f"ph{m}_{n}")
                       for n in range(NCHUNK)]
                seq = m == 0 or m == MT - 1
                if seq:
                    # sequential chains: m0 - only s8 half 0 resident early;
                    # m15 - drain of chain 0 overlaps chain 1
                    for n in range(NCHUNK):
                        for kc in range(KC):
                            if n == 0 and kc == 10 and m + 1 < MT:
                                a_prev = prep(m + 1, 2)
                            mm(pts[n], a_pack, n, kc)
                        drain(n, r_sb, pts[n], m, dma_now=True)
                else:
                    # interleaved: both chains share the stationary a'(kc)
                    for kc in range(KC):
                        if kc == KC // 2 and m + 1 < MT:
                            a_prev = prep(m + 1, 2)
                        for n in range(NCHUNK):
                            mm(pts[n], a_pack, n, kc)
                    for n in range(NCHUNK):
                        drain(n, r_sb, pts[n], m, dma_now=False)
                    nc.gpsimd.dma_start(
                        r_d[m * 128:(m + 1) * 128, :], r_sb
                    )

    nc.compile()
    return nc


def kernel(x: np.ndarray, w: np.ndarray) -> np.ndarray:
    import ml_dtypes
    from concourse.bass_utils import run_bass_kernel_spmd

    assert x.shape == (BATCH, N_IN) and w.shape == (N_IN, N_OUT)
    x = np.ascontiguousarray(x, dtype=np.float32)
    w = np.ascontiguousarray(w, dtype=np.float32)

    if "nc" not in _CACHED:
        _CACHED["nc"] = _build_nc()
    nc = _CACHED["nc"]

    # static weight preprocessing: sign(clip(w)), exactly representable
    s_full = np.sign(np.clip(w, -1.0, 1.0)).astype(np.float32)
    s8_full = s_full.astype(ml_dtypes.float8_e4m3)
    xu8 = np.rint(x * XQ).astype(np.uint8)

    xt_by_bg = {}
    s8_by_ng = {}
    for bg in range(B_GROUPS):
        xsh = xu8[bg * B_SH:(bg + 1) * B_SH, :]
        # [k, b] -> [m, p, kc, i, mb] with k = kc*256 + i*128 + p
        xt_by_bg[bg] = np.ascontiguousarray(
            xsh.T.reshape(KC, 2, 128, MT, 128).transpose(3, 2, 0, 1, 4)
        )
    for ng in range(N_GROUPS):
        ssh = s8_full[:, ng * N_SH:(ng + 1) * N_SH]
        # [k, n] -> [nch, p, kc, i, nn]
        s8_by_ng[ng] = np.ascontiguousarray(
            ssh.reshape(KC, 2, 128, NCHUNK, 512).transpose(3, 2, 0, 1, 4)
        )

    in_maps = []
    for c in range(8):
        bg, ng = divmod(c, N_GROUPS)
        in_maps.append({"xt": xt_by_bg[bg], "s8": s8_by_ng[ng]})

    trace = os.environ.get("PH_KERNEL_TRACE", "") == "1"
    kwargs = {"trace_cores": list(range(8))} if trace else {}
    try:
        res = run_bass_kernel_spmd(
            nc, in_maps, core_ids=list(range(8)), trace=trace, **kwargs
        )
    except Exception as e:  # transient NRT_EXEC_UNIT_UNRECOVERABLE seen rarely
        if "UNRECOVERABLE" not in str(e) and "UNAVAILABLE" not in str(e):
            raise
        import time
        time.sleep(5.0)
        res = run_bass_kernel_spmd(
            nc, in_maps, core_ids=list(range(8)), trace=trace, **kwargs
        )
    if trace:
        _CACHED["last_result"] = res

    # gather device partial r' = (d8 @ s) from the cores, add the exact
    # centering shift CENTER * colsum(s) back in
    rp = np.empty((BATCH, N_OUT), dtype=np.float32)
    for c, r in enumerate(res.results):
        bg, ng = divmod(c, N_GROUPS)
        rp[bg * B_SH:(bg + 1) * B_SH, ng * N_SH:(ng + 1) * N_SH] = (
            r["r16"].astype(np.float32)
        )
    _CACHED["rp_raw"] = rp.copy()
    colsum = s_full.sum(axis=0, dtype=np.float64)  # exact small integers
    rp = rp.astype(np.float64) + CENTER * colsum[None, :]
    r_full = rp / RSCALE

    # ---- host refinement of the near-cancellation band |r| < T_BAND:
    # recompute r = 0.1 * h . s exactly (products are +-h, exact in f32;
    # pairwise f64 sum), grouped by row to keep gather traffic low.
    flag = np.abs(r_full) < T_BAND
    if flag.any():
        h32 = np.power(10.0, -x.astype(np.float64)).astype(np.float32)
        sT8 = np.ascontiguousarray(np.sign(w).T.astype(np.int8))
        rows = np.nonzero(flag.any(axis=1))[0]
        for b in rows:
            cols = np.nonzero(flag[b])[0]
            prod = h32[b][None, :] * sT8[cols]
            r_full[b, cols] = prod.sum(axis=1, dtype=np.float64) * 0.1

    # ---- host log epilogue: ph = 7 + sign(r)*(-log10(|r|/409.6) - 7)
    t = np.abs(r_full) + 1e-300
    u = -np.log10(t) + (np.log10(409.6) - 7.0)
    y = np.where(r_full < 0, -u, u) + 7.0
    return y.astype(np.float32)
